# revision 35
# baseline (speedup 1.0000x reference)
"""Trainium2 Bass kernel for nn_Attention_84473416778449.

Reference computation (B=2, S=2048, D=1024, H=16, HD=64, fp32):
    q/k/v = x @ w{q,k,v}.T ; RoPE(q, k) ; causal softmax attention ; out @ wo.T

Sharding: 8 cores = (batch 2) x (head-group 4). Each core computes 4 heads of
one batch end-to-end and a partial output projection over its 256 channels;
the host sums the 4 partials per batch (in fp32; device emits fp16 partials).

v2 structure (vs the 222us baseline):
  * All inputs pre-cast to fp16 on the host; loads go out as fine-grained
    HWDGE DMAs (x + out stores on sync, weights/tables on scalar) so the
    first projection matmul issues ~2us in and converting-DMA slow paths
    are gone entirely.
  * Score matmuls for the two heads of a tile pair are packed into disjoint
    PE row-groups (tile_position (0,0)/(64,0)) and run concurrently.
  * Each attention wave = one key tile x head pair -> one [128,1024] PSUM
    group (2 banks); exp is a single merged ACTIVATE over both heads
    (halves ACT instruction-overhead); causal masking is one 2-D
    affine_select over the merged fp16 probs tile on gpsimd.
  * Waves are software-pipelined (scores(i+1) -> exp(i) -> filler -> PV(i))
    with the next block's projections and the previous block's output
    projections spliced in as PE filler, so the PE never idles long enough
    for the HAM clock gate to re-throttle.
  * RoPE: proj result is evacuated once to fp16 SBUF (DVE), the cos/sin
    multiplies run on gpsimd, the rotate-half runs on the PE (rt2 matmul),
    and only the final add stays on the DVE.
  * Softmax normalize uses reciprocal_approx_fast (1 DVE op, ~51 ULP).
  * The last block's output projections are split into head-pair halves so
    the hp0 half runs as filler inside the final attention block.
PSUM budget: scores 2x[128,1024] (4 banks) + PV accumulators 2x[128,512]
(2 banks) + proj/rope/out rotation 2x[128,512] (2 banks) = 8 banks.
"""
import sys

if "/opt/trn_rl_repo" not in sys.path:
    sys.path.insert(0, "/opt/trn_rl_repo")

import numpy as np

import concourse.bass as bass
import concourse.mybir as mybir
import concourse.tile as tile
from concourse import bacc
from concourse.bass_utils import run_bass_kernel_spmd

B, S, D, H, HD = 2, 2048, 1024, 16, 64
NCORES = 8
GROUPS = 4            # head groups
GH = H // GROUPS      # heads per group = 4
GC = GH * HD          # channels per group = 256
KT = D // 128         # 8 k-tiles over D
ST = S // 128         # 16 s-tiles
QB = 4                # sq blocks of 512
QW = S // QB          # 512
VW = GH * (HD + 1)    # 260: v tile payload columns

f32 = mybir.dt.float32
f16 = mybir.dt.float16
MMDT = mybir.dt.float16
Exp = mybir.ActivationFunctionType.Exp
Copy = mybir.ActivationFunctionType.Copy
Mult = mybir.AluOpType.mult
Add = mybir.AluOpType.add

_cache = {}


def _build():
    nc = bacc.Bacc("TRN2", num_devices=NCORES)

    xT = nc.dram_tensor("xT", [D, S], f16, kind="ExternalInput").ap()
    wqT = nc.dram_tensor("wqT", [128, KT * GC], f16, kind="ExternalInput").ap()
    wkT = nc.dram_tensor("wkT", [128, KT * GC], f16, kind="ExternalInput").ap()
    wvT = nc.dram_tensor("wvT", [128, KT * GC], f16, kind="ExternalInput").ap()
    woT = nc.dram_tensor("woT", [GC, D], f16, kind="ExternalInput").ap()
    cs2 = nc.dram_tensor("cs2", [128, S], f16, kind="ExternalInput").ap()
    sn2 = nc.dram_tensor("sn2", [128, S], f16, kind="ExternalInput").ap()
    out = nc.dram_tensor("out", [S, D], f16, kind="ExternalOutput").ap()

    with tile.TileContext(nc) as tc:
        with tc.tile_pool(name="persist", bufs=1) as pp, \
             tc.tile_pool(name="rope", bufs=3) as rp, \
             tc.tile_pool(name="probs", bufs=4) as wp, \
             tc.tile_pool(name="outsb", bufs=3) as op_, \
             tc.tile_pool(name="small", bufs=3) as sp:

            # ---- persistent SBUF tiles ------------------------------------
            # one [128, S] tile per k-tile: a single DMA loads all 4 blocks
            xTw = [pp.tile([128, S], MMDT, tag=f"xT{kt}", name=f"xTw{kt}")
                   for kt in range(KT)]
            xTr = [[xTw[kt][:, cb * QW:(cb + 1) * QW] for cb in range(QB)]
                   for kt in range(KT)]
            wq_s = pp.tile([128, KT * GC], MMDT, tag="wq", name="wq_s")
            wk_s = pp.tile([128, KT * GC], MMDT, tag="wk", name="wk_s")
            wv_s = pp.tile([128, KT * GC], MMDT, tag="wv", name="wv_s")
            wo_s = [pp.tile([128, D], MMDT, tag=f"wo{kt}", name=f"wo{kt}")
                    for kt in range(2)]
            cs_sb = pp.tile([128, S], f16, tag="cs")
            sn_sb = pp.tile([128, S], f16, tag="sn")

            # ---- loads: x k-tiles split across both HWDGE rings and
            # interleaved with the weight chunks in first-use order, so the
            # P(0) projection chains never starve on a single ring ----------
            def ld_x(eng, kt):
                eng.dma_start(xTw[kt][:], xT[kt * 128:(kt + 1) * 128, :])

            nc.scalar.dma_start(wq_s[:, 0:2 * GC], wqT[:, 0:2 * GC])
            for kt in (0, 1, 2, 3):
                ld_x(nc.sync, kt)
            ld_x(nc.scalar, 4)
            nc.scalar.dma_start(wq_s[:, 2 * GC:], wqT[:, 2 * GC:])
            ld_x(nc.scalar, 5)
            nc.sync.dma_start(cs_sb[:, 0:QW], cs2[:, 0:QW])
            nc.sync.dma_start(sn_sb[:, 0:QW], sn2[:, 0:QW])
            nc.scalar.dma_start(wk_s[:, 0:2 * GC], wkT[:, 0:2 * GC])
            ld_x(nc.sync, 6)
            nc.scalar.dma_start(wk_s[:, 2 * GC:], wkT[:, 2 * GC:])
            ld_x(nc.scalar, 7)
            nc.sync.dma_start(wv_s[:, 0:4 * GC], wvT[:, 0:4 * GC])
            nc.sync.dma_start(wv_s[:, 4 * GC:], wvT[:, 4 * GC:])
            nc.scalar.dma_start(cs_sb[:, QW:], cs2[:, QW:])
            nc.scalar.dma_start(sn_sb[:, QW:], sn2[:, QW:])
            for kt in range(2):
                nc.sync.dma_start(wo_s[kt][:], woT[kt * 128:(kt + 1) * 128, :])

            # ---- constants (rotate-half matrix, built fp32 -> cast) -------
            cscratch = pp.tile([128, 128], f32, tag="cscratch")
            nc.gpsimd.memset(cscratch[:], 0.0)
            for blk in range(2):
                sub = cscratch[blk * 64:(blk + 1) * 64,
                               blk * 64:(blk + 1) * 64]
                nc.gpsimd.affine_select(   # -1 where p - f == 32
                    out=sub, in_=sub, pattern=[[-1, 64]], base=-32,
                    channel_multiplier=1,
                    compare_op=mybir.AluOpType.not_equal, fill=-1.0)
                nc.gpsimd.affine_select(   # +1 where f - p == 32
                    out=sub, in_=sub, pattern=[[1, 64]], base=-32,
                    channel_multiplier=-1,
                    compare_op=mybir.AluOpType.not_equal, fill=1.0)
            rt2 = pp.tile([128, 128], MMDT, tag="rt2")
            nc.vector.tensor_copy(rt2[:], cscratch[:])

            ones65 = pp.tile([65, 64], MMDT, tag="ones65")
            nc.gpsimd.memset(ones65[:], 1.0)

            qT = [[pp.tile([128, QW], MMDT, tag=f"qT{i}_{b}",
                           name=f"qT{i}_{b}") for b in range(QB)]
                  for i in range(2)]
            kTt = [[pp.tile([128, QW], MMDT, tag=f"kT{i}_{b}",
                            name=f"kT{i}_{b}") for b in range(QB)]
                   for i in range(2)]
            attnT = [[pp.tile([128, QW], MMDT, tag=f"aT{i}_{b}",
                              name=f"aT{i}_{b}") for b in range(QB)]
                     for i in range(2)]
            v_sb = [pp.tile([128, VW + 64], MMDT, tag=f"v{i}",
                            name=f"v{i}") for i in range(ST)]
            # fp32 partials for the split last-block output projection
            ohalf = [pp.tile([128, QW], f32, tag=f"oh{i}", name=f"oh{i}")
                     for i in range(8)]

            with tc.tile_pool(name="psS", bufs=2, space="PSUM") as psS, \
                 tc.tile_pool(name="psO", bufs=2, space="PSUM") as psO, \
                 tc.tile_pool(name="psA", bufs=2, space="PSUM") as psA:

                def qk_proj_granules(w_src, dst, hp, sb):
                    # g12 is atomic: the full PSUM accumulation chain plus
                    # its evacuation stay inside one granule so no other
                    # psA "mix" allocation can rotate into the open slot.
                    # Only the rope matmul (g3) is split out, so it never
                    # head-of-line blocks the PE queue on the gpsimd
                    # multiplies it depends on.
                    cols = slice(sb * QW, (sb + 1) * QW)
                    st_ = {}

                    def g12():
                        pq = psA.tile([128, QW], f32, tag="mix",
                                      name=f"pq_{hp}_{sb}")
                        for kt in range(KT):
                            nc.tensor.matmul(
                                pq[:],
                                w_src[:, kt * GC + hp * 128:
                                      kt * GC + hp * 128 + 128],
                                xTr[kt][sb],
                                start=(kt == 0), stop=(kt == KT - 1))
                        qraw = rp.tile([128, QW], MMDT, tag="qraw")
                        nc.vector.tensor_copy(qraw[:], pq[:])
                        tsn = rp.tile([128, QW], MMDT, tag="tsn")
                        nc.gpsimd.tensor_tensor(
                            out=tsn[:], in0=qraw[:], in1=sn_sb[:, cols],
                            op=Mult)
                        tcs = rp.tile([128, QW], MMDT, tag="tcs")
                        nc.gpsimd.tensor_tensor(
                            out=tcs[:], in0=qraw[:], in1=cs_sb[:, cols],
                            op=Mult)
                        st_["rope"] = (tsn, tcs)

                    def g3():
                        tsn, tcs = st_.pop("rope")
                        pr = psA.tile([128, QW], f32, tag="mix",
                                      name=f"pr_{hp}_{sb}")
                        nc.tensor.matmul(pr[:], rt2[:], tsn[:],
                                         start=True, stop=True)
                        nc.vector.tensor_tensor(
                            out=dst[hp][sb][:], in0=pr[:], in1=tcs[:],
                            op=Add)

                    return [(1720, g12), (260, g3)]

                def emit_qk_proj(w_src, dst, hp, sb):
                    for _, g in qk_proj_granules(w_src, dst, hp, sb):
                        g()

                def emit_v(st):
                    pv = psA.tile([128, QW], f32, tag="mix",
                                  name=f"pv_{st}")
                    for kt in range(KT):
                        nc.tensor.matmul(
                            pv[:, 0:GC],
                            xTr[kt][st // 4][:, (st % 4) * 128:
                                             (st % 4) * 128 + 128],
                            wv_s[:, kt * GC:(kt + 1) * GC],
                            start=(kt == 0), stop=(kt == KT - 1))
                    vt = v_sb[st]
                    vhe = vt[:, 0:VW].rearrange("p (h e) -> p h e", e=HD + 1)
                    nc.gpsimd.memset(vt[:, VW:VW + 64], 0.0)
                    nc.gpsimd.memset(vhe[:, :, HD:HD + 1], 1.0)
                    nc.vector.tensor_copy(
                        vhe[:, :, 0:HD],
                        pv[:, 0:GC].rearrange("p (h d) -> p h d", d=HD))

                def out_granules(st):
                    # two granules (one per 512-col half); store rides on the
                    # second so the whole row goes out as one DMA
                    st_ = {}

                    def g(db):
                        if db == 0:
                            st_["ob"] = op_.tile([128, D], f16, tag="outsb",
                                                 name=f"ob_{st}")
                        ob = st_["ob"]
                        pc = psA.tile([128, QW], f32, tag="mix",
                                      name=f"pc_{st}_{db}")
                        for kt in range(2):
                            nc.tensor.matmul(
                                pc[:],
                                attnT[kt][st // 4][:, (st % 4) * 128:
                                                   (st % 4) * 128 + 128],
                                wo_s[kt][:, db * QW:(db + 1) * QW],
                                start=(kt == 0), stop=(kt == 1))
                        nc.vector.tensor_copy(
                            ob[:, db * QW:(db + 1) * QW], pc[:])
                        if db == 1:
                            st_.pop("ob")
                            nc.sync.dma_start(
                                out[st * 128:(st + 1) * 128, :], ob[:])

                    return [(480, lambda: g(0)), (520, lambda: g(1))]

                def emit_out(st):
                    for _, g in out_granules(st):
                        g()

                def warmer():
                    # dummy 512-col matmul: keeps the HAM clock gate at 8/8
                    # through ACT-paced stretches with no real filler left
                    pw = psA.tile([128, QW], f32, tag="mix", name="warm")
                    nc.tensor.matmul(pw[:], rt2[:], xTw[0][:, 0:QW],
                                     start=True, stop=True)

                def emit_out_half0(st, db):
                    # hp0 half-contraction of the block-3 output projection
                    pc = psA.tile([128, QW], f32, tag="mix",
                                  name=f"pc0_{st}_{db}")
                    nc.tensor.matmul(
                        pc[:],
                        attnT[0][st // 4][:, (st % 4) * 128:
                                          (st % 4) * 128 + 128],
                        wo_s[0][:, db * QW:(db + 1) * QW],
                        start=True, stop=True)
                    oh = ohalf[(st % 4) * 2 + db]
                    nc.vector.tensor_copy(oh[:], pc[:])

                def emit_out_half1(st):
                    ob = op_.tile([128, D], f16, tag="outsb")
                    for db in range(2):
                        pc = psA.tile([128, QW], f32, tag="mix",
                                      name=f"pc1_{st}_{db}")
                        nc.tensor.matmul(
                            pc[:],
                            attnT[1][st // 4][:, (st % 4) * 128:
                                              (st % 4) * 128 + 128],
                            wo_s[1][:, db * QW:(db + 1) * QW],
                            start=True, stop=True)
                        nc.vector.tensor_tensor(
                            out=ob[:, db * QW:(db + 1) * QW], in0=pc[:],
                            in1=ohalf[(st % 4) * 2 + db][:], op=Add)
                    nc.sync.dma_start(out[st * 128:(st + 1) * 128, :], ob[:])

                def normalize_evac(hp, qb, hh, po):
                    # one fp16 copy frees the psO slot immediately so the
                    # next half-block's PV chain is not blocked
                    poc = sp.tile([65, QW], MMDT, tag="poc",
                                  name=f"poc_{hp}_{qb}_{hh}")
                    nc.vector.tensor_copy(poc[:], po[0:HD + 1, :])
                    return poc

                def normalize_tail(hp, qb, hh, poc):
                    # denominator broadcast is a tiny K=1 PE matmul off
                    # poc's row 64; deferred ~2 waves so it never head-of-
                    # line blocks the PE queue on the DVE poc copy
                    pb = hh * 64
                    dn = psA.tile([64, QW], f32, tag="mix",
                                  name=f"dn_{hp}_{qb}_{hh}")
                    nc.tensor.matmul(dn[:], ones65[HD:HD + 1, :],
                                     poc[HD:HD + 1, :],
                                     start=True, stop=True)
                    rcb = sp.tile([64, QW], f32, tag="recb")
                    nc.vector.reciprocal_approx_fast(out=rcb[:], in_=dn[:])
                    nc.vector.tensor_tensor(
                        out=attnT[hp][qb][pb:pb + 64, :],
                        in0=poc[0:HD, :], in1=rcb[:], op=Mult)

                def emit_attention(qb, fillers, late_fillers=(),
                                   span_f=None):
                    """Waves over (hp, kt); scores(i+1)->exp(i)->filler->PV(i).

                    fillers: closures to splice in during early waves
                    late_fillers: spliced during the hp=1 half (block 3)
                    """
                    nsk = (qb + 1) * 4
                    # build the flat wave list
                    waves = [(hp, kt) for hp in range(2) for kt in range(nsk)]
                    nW = len(waves)
                    fillers = list(fillers)
                    late = list(late_fillers)
                    # early fillers must land in the first 3/4 of waves of
                    # each half so block-3 k/v projections finish before the
                    # diagonal waves need them
                    po_t = {}
                    sc_t = {}
                    prt_t = {}

                    def emit_scores(i):
                        hp, kt = waves[i]
                        if kt == 0:
                            for hh in range(2):
                                po_t[(hp, hh)] = psO.tile(
                                    [128, QW], f32, tag="pvacc",
                                    name=f"po_{hp}_{hh}_{qb}")
                        c0 = max(0, kt * 128 - qb * QW)
                        ps = psS.tile([128, 2 * QW], f32, tag="spair",
                                      name=f"sc_{hp}_{qb}_{kt}")
                        sc_t[i] = (ps, c0)
                        for hh in range(2):
                            pb = hh * 64
                            nc.tensor.matmul(
                                ps[:, hh * QW + c0:(hh + 1) * QW],
                                kTt[hp][kt // 4][pb:pb + 64,
                                                 (kt % 4) * 128:
                                                 (kt % 4) * 128 + 128],
                                qT[hp][qb][pb:pb + 64, c0:QW],
                                start=True, stop=True,
                                tile_position=(pb, 0))

                    def emit_exp(i):
                        hp, kt = waves[i]
                        ps, c0 = sc_t[i]
                        prt = wp.tile([128, 2 * QW], MMDT, tag="probs",
                                      name=f"pr_{hp}_{qb}_{kt}")
                        prt_t[i] = prt
                        nc.scalar.activation(prt[:], ps[:], Exp, scale=0.125)
                        if kt >= nsk - 4:
                            j = kt - (nsk - 4)
                            nc.gpsimd.affine_select(
                                out=prt[:].rearrange(
                                    "p (t f) -> p t f", f=QW),
                                in_=prt[:].rearrange(
                                    "p (t f) -> p t f", f=QW),
                                pattern=[[0, 2], [1, QW]], base=-128 * j,
                                channel_multiplier=-1,
                                compare_op=mybir.AluOpType.is_ge,
                                fill=0.0)

                    def emit_pv(i, now=None):
                        now = i if now is None else now
                        hp, kt = waves[i]
                        prt = prt_t.pop(i)
                        _, c0 = sc_t.pop(i)
                        for hh in range(2):
                            h = 2 * hp + hh
                            nc.tensor.matmul(
                                po_t[(hp, hh)][:, c0:QW],
                                v_sb[kt][:, h * (HD + 1):
                                         h * (HD + 1) + 128],
                                prt[:, hh * QW + c0:(hh + 1) * QW],
                                start=(kt == 0), stop=(kt == nsk - 1))
                        if kt == nsk - 1:
                            for hh in range(2):
                                poc = normalize_evac(
                                    hp, qb, hh, po_t.pop((hp, hh)))
                                deferred.setdefault(now + 1, []).append(
                                    lambda h1=hp, h2=hh, p=poc:
                                    normalize_tail(h1, qb, h2, p))

                    # software pipeline; filler granules are (cost_ns,
                    # closure), paced so cumulative filler PE-time tracks the
                    # wave index (fillers over the first span_f waves, late
                    # fillers over the first 3/4 of the hp=1 half). When the
                    # pools run dry mid-block, dummy warmers keep the PE duty
                    # cycle high enough that the HAM clock gate stays at 8/8.
                    if span_f is None:
                        span_f = max(1, (3 * nW) // 4)
                    span_l = max(1, (3 * nsk) // 4)
                    tot_f = sum(c for c, _ in fillers) or 1
                    tot_l = sum(c for c, _ in late) or 1
                    deferred = {}
                    emit_scores(0)
                    cf = cl = 0.0
                    fi = li = 0
                    for i in range(nW):
                        if i + 1 < nW:
                            emit_scores(i + 1)
                        emit_exp(i)
                        for g in deferred.pop(i, []):
                            g()
                        popped = 0
                        tgt = tot_f * min(i + 1, span_f) / span_f
                        while fi < len(fillers) and cf < tgt:
                            c, g = fillers[fi]
                            g()
                            cf += c
                            fi += 1
                            popped += 1
                        # late fillers start 2 waves into the hp=1 half so
                        # the hp=0 deferred normalize tails (which write
                        # attnT for this block) are emitted first
                        j = i - nsk - 3
                        if j >= 0 and late:
                            tgt = tot_l * min(j + 1, span_l) / span_l
                            while li < len(late) and cl < tgt:
                                c, g = late[li]
                                g()
                                cl += c
                                li += 1
                                popped += 1
                        if i > 1:
                            emit_pv(i - 2, i)
                    emit_pv(nW - 2, nW)
                    emit_pv(nW - 1, nW)
                    for k in sorted(deferred):
                        for g in deferred.pop(k):
                            g()
                    for _, g in fillers[fi:] + late[li:]:
                        g()

                # ================= main schedule ==========================
                def p_granules(sb, which="qkv"):
                    # interleave chains: [c0g1 c0g2 c1g1 c1g2 c0g3 c2g1
                    # c2g2 c1g3 ...] so each rope matmul (g3) trails its
                    # gpsimd inputs by >=2 granules of other PE work and at
                    # most two psA "mix" slots are ever live
                    chains = []
                    if "q" in which:
                        for hp in range(2):
                            chains.append(qk_proj_granules(wq_s, qT, hp, sb))
                    if "k" in which:
                        for hp in range(2):
                            chains.append(qk_proj_granules(wk_s, kTt, hp, sb))
                    vg = []
                    if "v" in which:
                        vg = [(880, lambda s=st: emit_v(s))
                              for st in range(4 * sb, 4 * sb + 4)]
                    seq = []
                    prev3 = None
                    for ch in chains:
                        seq.append(ch[0])
                        if prev3 is not None:
                            seq.append(prev3)
                        prev3 = ch[1]
                    for i, g in enumerate(vg):
                        seq.append(g)
                        if i == 0 and prev3 is not None:
                            seq.append(prev3)
                            prev3 = None
                    if prev3 is not None:
                        seq.append(prev3)
                    return seq

                # P(0)
                for _, g in p_granules(0):
                    g()

                # A(0) with P(1) as filler
                emit_attention(0, p_granules(1))

                # A(1): P(2) as filler (out-projections deferred to later
                # blocks, which have no projection work left)
                emit_attention(1, p_granules(2))

                # A(2): P(3) q-projs + O(0) + O(1) as filler
                f = p_granules(3, "q")
                for st in range(0, 8):
                    f += out_granules(st)
                emit_attention(2, f)

                # A(3): P(3) k/v projs + O(2) during hp=0 (k/v done before
                # the diagonal waves need them); block-3 hp0-half outs
                # during the hp=1 waves
                f = p_granules(3, "kv")
                for st in range(8, 12):
                    f += out_granules(st)
                late = [(260, lambda s=st, d=db: emit_out_half0(s, d))
                        for st in range(12, 16) for db in range(2)]
                emit_attention(3, f, late, span_f=12)

                # tail: hp1 halves + add + store
                for st in range(12, 16):
                    emit_out_half1(st)

    nc.compile()
    return nc


def _shard_inputs(x, cos, sin, wq, wk, wv, wo):
    x = np.asarray(x, dtype=np.float32)
    cosT = np.asarray(cos, np.float32).reshape(S, HD).T
    sinT = np.asarray(sin, np.float32).reshape(S, HD).T
    cs2 = np.ascontiguousarray(
        np.concatenate([cosT, cosT], axis=0)).astype(np.float16)
    sn2 = np.ascontiguousarray(
        np.concatenate([sinT, sinT], axis=0)).astype(np.float16)

    def prep_w(w, rows):
        # [GC, D] row slice -> transpose -> [D, GC] -> per-128 k-chunks to
        # [128, KT*GC] so every on-device slice is a contiguous DMA
        wT = np.asarray(w, np.float32)[rows, :].T          # [D, GC]
        return np.ascontiguousarray(
            wT.reshape(KT, 128, GC).transpose(1, 0, 2).reshape(128, KT * GC)
        ).astype(np.float16)

    in_maps = []
    for c in range(NCORES):
        b, g = c // GROUPS, c % GROUPS
        rows = slice(g * GC, (g + 1) * GC)
        in_maps.append({
            "xT": np.ascontiguousarray(x[b].T).astype(np.float16),
            "wqT": prep_w(wq, rows),
            "wkT": prep_w(wk, rows),
            "wvT": prep_w(wv, rows),
            "woT": np.ascontiguousarray(
                np.asarray(wo, np.float32)[:, rows].T).astype(np.float16),
            "cs2": cs2,
            "sn2": sn2,
        })
    return in_maps


def _run(inputs, trace=False, trace_kwargs=None):
    if "nc" not in _cache:
        _cache["nc"] = _build()
    nc = _cache["nc"]
    in_maps = _shard_inputs(
        inputs["x"], inputs["cos"], inputs["sin"],
        inputs["wq"], inputs["wk"], inputs["wv"], inputs["wo"])
    res = run_bass_kernel_spmd(
        nc, in_maps, list(range(NCORES)), trace=trace,
        **(trace_kwargs or {}))
    full = np.zeros((B, S, D), dtype=np.float32)
    for c in range(NCORES):
        full[c // GROUPS] += res.results[c]["out"].astype(np.float32)
    return full, res


def kernel(**inputs):
    full, _ = _run(inputs, trace=False)
    return full


# revision 37
# speedup vs baseline: 1.1525x; 1.1525x over previous
"""Trainium2 Bass kernel for nn_Attention_84473416778449.

Reference computation (B=2, S=2048, D=1024, H=16, HD=64, fp32):
    q/k/v = x @ w{q,k,v}.T ; RoPE(q, k) ; causal softmax attention ; out @ wo.T

Sharding: 8 cores = (batch 2) x (head-group 4). Each core computes 4 heads of
one batch end-to-end and a partial output projection over its 256 channels;
the host sums the 4 partials per batch (in fp32; device emits fp16 partials).

v2 structure (vs the 222us baseline):
  * All inputs pre-cast to fp16 on the host; loads go out as fine-grained
    HWDGE DMAs (x + out stores on sync, weights/tables on scalar) so the
    first projection matmul issues ~2us in and converting-DMA slow paths
    are gone entirely.
  * Score matmuls for the two heads of a tile pair are packed into disjoint
    PE row-groups (tile_position (0,0)/(64,0)) and run concurrently.
  * Each attention wave = one key tile x head pair -> one [128,1024] PSUM
    group (2 banks); exp is a single merged ACTIVATE over both heads
    (halves ACT instruction-overhead); causal masking is one 2-D
    affine_select over the merged fp16 probs tile on gpsimd.
  * Waves are software-pipelined (scores(i+1) -> exp(i) -> filler -> PV(i))
    with the next block's projections and the previous block's output
    projections spliced in as PE filler, so the PE never idles long enough
    for the HAM clock gate to re-throttle.
  * RoPE: proj result is evacuated once to fp16 SBUF (DVE), the cos/sin
    multiplies run on gpsimd, the rotate-half runs on the PE (rt2 matmul),
    and only the final add stays on the DVE.
  * Softmax normalize uses reciprocal_approx_fast (1 DVE op, ~51 ULP).
  * The last block's output projections are split into head-pair halves so
    the hp0 half runs as filler inside the final attention block.
PSUM budget: scores 2x[128,1024] (4 banks) + PV accumulators 2x[128,512]
(2 banks) + proj/rope/out rotation 2x[128,512] (2 banks) = 8 banks.
"""
import sys

if "/opt/trn_rl_repo" not in sys.path:
    sys.path.insert(0, "/opt/trn_rl_repo")

import numpy as np

import concourse.bass as bass
import concourse.mybir as mybir
import concourse.tile as tile
from concourse import bacc
from concourse.bass_utils import run_bass_kernel_spmd

B, S, D, H, HD = 2, 2048, 1024, 16, 64
NCORES = 8
GROUPS = 4            # head groups
GH = H // GROUPS      # heads per group = 4
GC = GH * HD          # channels per group = 256
KT = D // 128         # 8 k-tiles over D
ST = S // 128         # 16 s-tiles
QB = 4                # sq blocks of 512
QW = S // QB          # 512
VW = GH * (HD + 1)    # 260: v tile payload columns

f32 = mybir.dt.float32
f16 = mybir.dt.float16
MMDT = mybir.dt.float16
Exp = mybir.ActivationFunctionType.Exp
Copy = mybir.ActivationFunctionType.Copy
Mult = mybir.AluOpType.mult
Add = mybir.AluOpType.add

_cache = {}


def _build():
    nc = bacc.Bacc("TRN2", num_devices=NCORES)

    xT = nc.dram_tensor("xT", [D, S], f16, kind="ExternalInput").ap()
    wqT = nc.dram_tensor("wqT", [128, KT * GC], f16, kind="ExternalInput").ap()
    wkT = nc.dram_tensor("wkT", [128, KT * GC], f16, kind="ExternalInput").ap()
    wvT = nc.dram_tensor("wvT", [128, KT * GC], f16, kind="ExternalInput").ap()
    woT = nc.dram_tensor("woT", [GC, D], f16, kind="ExternalInput").ap()
    cs2 = nc.dram_tensor("cs2", [128, S], f16, kind="ExternalInput").ap()
    sn2 = nc.dram_tensor("sn2", [128, S], f16, kind="ExternalInput").ap()
    out = nc.dram_tensor("out", [S, D], f16, kind="ExternalOutput").ap()

    with tile.TileContext(nc) as tc:
        with tc.tile_pool(name="persist", bufs=1) as pp, \
             tc.tile_pool(name="rope", bufs=3) as rp, \
             tc.tile_pool(name="probs", bufs=4) as wp, \
             tc.tile_pool(name="outsb", bufs=3) as op_, \
             tc.tile_pool(name="small", bufs=3) as sp:

            # ---- persistent SBUF tiles ------------------------------------
            # one [128, S] tile per k-tile: a single DMA loads all 4 blocks
            xTw = [pp.tile([128, S], MMDT, tag=f"xT{kt}", name=f"xTw{kt}")
                   for kt in range(KT)]
            xTr = [[xTw[kt][:, cb * QW:(cb + 1) * QW] for cb in range(QB)]
                   for kt in range(KT)]
            wq_s = pp.tile([128, KT * GC], MMDT, tag="wq", name="wq_s")
            wk_s = pp.tile([128, KT * GC], MMDT, tag="wk", name="wk_s")
            wv_s = pp.tile([128, KT * GC], MMDT, tag="wv", name="wv_s")
            wo_s = [pp.tile([128, D], MMDT, tag=f"wo{kt}", name=f"wo{kt}")
                    for kt in range(2)]
            cs_sb = pp.tile([128, S], f16, tag="cs")
            sn_sb = pp.tile([128, S], f16, tag="sn")

            # ---- loads: x k-tiles split across both HWDGE rings and
            # interleaved with the weight chunks in first-use order, so the
            # P(0) projection chains never starve on a single ring ----------
            def ld_x(eng, kt):
                eng.dma_start(xTw[kt][:], xT[kt * 128:(kt + 1) * 128, :])

            nc.scalar.dma_start(wq_s[:, 0:2 * GC], wqT[:, 0:2 * GC])
            for kt in (0, 1, 2, 3):
                ld_x(nc.sync, kt)
            ld_x(nc.scalar, 4)
            nc.scalar.dma_start(wq_s[:, 2 * GC:], wqT[:, 2 * GC:])
            ld_x(nc.scalar, 5)
            nc.sync.dma_start(cs_sb[:, 0:QW], cs2[:, 0:QW])
            nc.sync.dma_start(sn_sb[:, 0:QW], sn2[:, 0:QW])
            nc.scalar.dma_start(wk_s[:, 0:2 * GC], wkT[:, 0:2 * GC])
            ld_x(nc.sync, 6)
            nc.scalar.dma_start(wk_s[:, 2 * GC:], wkT[:, 2 * GC:])
            ld_x(nc.scalar, 7)
            nc.sync.dma_start(wv_s[:, 0:4 * GC], wvT[:, 0:4 * GC])
            nc.sync.dma_start(wv_s[:, 4 * GC:], wvT[:, 4 * GC:])
            nc.scalar.dma_start(cs_sb[:, QW:], cs2[:, QW:])
            nc.scalar.dma_start(sn_sb[:, QW:], sn2[:, QW:])
            for kt in range(2):
                nc.sync.dma_start(wo_s[kt][:], woT[kt * 128:(kt + 1) * 128, :])

            # ---- constants (rotate-half matrix, built fp32 -> cast) -------
            cscratch = pp.tile([128, 128], f32, tag="cscratch")
            nc.gpsimd.memset(cscratch[:], 0.0)
            for blk in range(2):
                sub = cscratch[blk * 64:(blk + 1) * 64,
                               blk * 64:(blk + 1) * 64]
                nc.gpsimd.affine_select(   # -1 where p - f == 32
                    out=sub, in_=sub, pattern=[[-1, 64]], base=-32,
                    channel_multiplier=1,
                    compare_op=mybir.AluOpType.not_equal, fill=-1.0)
                nc.gpsimd.affine_select(   # +1 where f - p == 32
                    out=sub, in_=sub, pattern=[[1, 64]], base=-32,
                    channel_multiplier=-1,
                    compare_op=mybir.AluOpType.not_equal, fill=1.0)
            rt2 = pp.tile([128, 128], MMDT, tag="rt2")
            nc.vector.tensor_copy(rt2[:], cscratch[:])

            ones65 = pp.tile([65, 64], MMDT, tag="ones65")
            nc.gpsimd.memset(ones65[:], 1.0)

            qT = [[pp.tile([128, QW], MMDT, tag=f"qT{i}_{b}",
                           name=f"qT{i}_{b}") for b in range(QB)]
                  for i in range(2)]
            kTt = [[pp.tile([128, QW], MMDT, tag=f"kT{i}_{b}",
                            name=f"kT{i}_{b}") for b in range(QB)]
                   for i in range(2)]
            attnT = [[pp.tile([128, QW], MMDT, tag=f"aT{i}_{b}",
                              name=f"aT{i}_{b}") for b in range(QB)]
                     for i in range(2)]
            v_sb = [pp.tile([128, VW + 64], MMDT, tag=f"v{i}",
                            name=f"v{i}") for i in range(ST)]
            # fp32 partials for the split last-block output projection
            ohalf = [pp.tile([128, QW], f32, tag=f"oh{i}", name=f"oh{i}")
                     for i in range(8)]

            with tc.tile_pool(name="psS", bufs=2, space="PSUM") as psS, \
                 tc.tile_pool(name="psO", bufs=2, space="PSUM") as psO, \
                 tc.tile_pool(name="psA", bufs=2, space="PSUM") as psA:

                def qk_proj_granules(w_src, dst, hp, sb):
                    # g12 is atomic: the full PSUM accumulation chain plus
                    # its evacuation stay inside one granule so no other
                    # psA "mix" allocation can rotate into the open slot.
                    # Only the rope matmul (g3) is split out, so it never
                    # head-of-line blocks the PE queue on the gpsimd
                    # multiplies it depends on.
                    cols = slice(sb * QW, (sb + 1) * QW)
                    st_ = {}

                    def g12():
                        pq = psA.tile([128, QW], f32, tag="mix",
                                      name=f"pq_{hp}_{sb}")
                        for kt in range(KT):
                            nc.tensor.matmul(
                                pq[:],
                                w_src[:, kt * GC + hp * 128:
                                      kt * GC + hp * 128 + 128],
                                xTr[kt][sb],
                                start=(kt == 0), stop=(kt == KT - 1))
                        qraw = rp.tile([128, QW], MMDT, tag="qraw")
                        nc.vector.tensor_copy(qraw[:], pq[:])
                        tsn = rp.tile([128, QW], MMDT, tag="tsn")
                        nc.gpsimd.tensor_tensor(
                            out=tsn[:], in0=qraw[:], in1=sn_sb[:, cols],
                            op=Mult)
                        tcs = rp.tile([128, QW], MMDT, tag="tcs")
                        nc.gpsimd.tensor_tensor(
                            out=tcs[:], in0=qraw[:], in1=cs_sb[:, cols],
                            op=Mult)
                        st_["rope"] = (tsn, tcs)

                    def g3():
                        tsn, tcs = st_.pop("rope")
                        pr = psA.tile([128, QW], f32, tag="mix",
                                      name=f"pr_{hp}_{sb}")
                        nc.tensor.matmul(pr[:], rt2[:], tsn[:],
                                         start=True, stop=True)
                        nc.vector.tensor_tensor(
                            out=dst[hp][sb][:], in0=pr[:], in1=tcs[:],
                            op=Add)

                    return [(1720, g12), (260, g3)]

                def emit_qk_proj(w_src, dst, hp, sb):
                    for _, g in qk_proj_granules(w_src, dst, hp, sb):
                        g()

                def emit_v(st):
                    pv = psA.tile([128, QW], f32, tag="mix",
                                  name=f"pv_{st}")
                    for kt in range(KT):
                        nc.tensor.matmul(
                            pv[:, 0:GC],
                            xTr[kt][st // 4][:, (st % 4) * 128:
                                             (st % 4) * 128 + 128],
                            wv_s[:, kt * GC:(kt + 1) * GC],
                            start=(kt == 0), stop=(kt == KT - 1))
                    vt = v_sb[st]
                    vhe = vt[:, 0:VW].rearrange("p (h e) -> p h e", e=HD + 1)
                    nc.gpsimd.memset(vt[:, VW:VW + 64], 0.0)
                    nc.gpsimd.memset(vhe[:, :, HD:HD + 1], 1.0)
                    nc.vector.tensor_copy(
                        vhe[:, :, 0:HD],
                        pv[:, 0:GC].rearrange("p (h d) -> p h d", d=HD))

                def out_granules(st):
                    # two granules (one per 512-col half); store rides on the
                    # second so the whole row goes out as one DMA
                    st_ = {}

                    def g(db):
                        if db == 0:
                            st_["ob"] = op_.tile([128, D], f16, tag="outsb",
                                                 name=f"ob_{st}")
                        ob = st_["ob"]
                        pc = psA.tile([128, QW], f32, tag="mix",
                                      name=f"pc_{st}_{db}")
                        for kt in range(2):
                            nc.tensor.matmul(
                                pc[:],
                                attnT[kt][st // 4][:, (st % 4) * 128:
                                                   (st % 4) * 128 + 128],
                                wo_s[kt][:, db * QW:(db + 1) * QW],
                                start=(kt == 0), stop=(kt == 1))
                        nc.vector.tensor_copy(
                            ob[:, db * QW:(db + 1) * QW], pc[:])
                        if db == 1:
                            st_.pop("ob")
                            nc.sync.dma_start(
                                out[st * 128:(st + 1) * 128, :], ob[:])

                    return [(480, lambda: g(0)), (520, lambda: g(1))]

                def emit_out(st):
                    for _, g in out_granules(st):
                        g()

                def warmer():
                    # dummy 512-col matmul: keeps the HAM clock gate at 8/8
                    # through ACT-paced stretches with no real filler left
                    pw = psA.tile([128, QW], f32, tag="mix", name="warm")
                    nc.tensor.matmul(pw[:], rt2[:], xTw[0][:, 0:QW],
                                     start=True, stop=True)

                def emit_out_half0(st, db):
                    # hp0 half-contraction of the block-3 output projection
                    pc = psA.tile([128, QW], f32, tag="mix",
                                  name=f"pc0_{st}_{db}")
                    nc.tensor.matmul(
                        pc[:],
                        attnT[0][st // 4][:, (st % 4) * 128:
                                          (st % 4) * 128 + 128],
                        wo_s[0][:, db * QW:(db + 1) * QW],
                        start=True, stop=True)
                    oh = ohalf[(st % 4) * 2 + db]
                    nc.vector.tensor_copy(oh[:], pc[:])

                def emit_out_half1(st):
                    ob = op_.tile([128, D], f16, tag="outsb")
                    for db in range(2):
                        pc = psA.tile([128, QW], f32, tag="mix",
                                      name=f"pc1_{st}_{db}")
                        nc.tensor.matmul(
                            pc[:],
                            attnT[1][st // 4][:, (st % 4) * 128:
                                              (st % 4) * 128 + 128],
                            wo_s[1][:, db * QW:(db + 1) * QW],
                            start=True, stop=True)
                        nc.vector.tensor_tensor(
                            out=ob[:, db * QW:(db + 1) * QW], in0=pc[:],
                            in1=ohalf[(st % 4) * 2 + db][:], op=Add)
                    nc.sync.dma_start(out[st * 128:(st + 1) * 128, :], ob[:])

                def normalize_evac(hp, qb, hh, po):
                    # one fp16 copy frees the psO slot immediately so the
                    # next half-block's PV chain is not blocked
                    poc = sp.tile([65, QW], MMDT, tag="poc",
                                  name=f"poc_{hp}_{qb}_{hh}")
                    nc.vector.tensor_copy(poc[:], po[0:HD + 1, :])
                    return poc

                def normalize_tail(hp, qb, hh, poc):
                    # denominator broadcast is a tiny K=1 PE matmul off
                    # poc's row 64; deferred ~2 waves so it never head-of-
                    # line blocks the PE queue on the DVE poc copy
                    pb = hh * 64
                    dn = psA.tile([64, QW], f32, tag="mix",
                                  name=f"dn_{hp}_{qb}_{hh}")
                    nc.tensor.matmul(dn[:], ones65[HD:HD + 1, :],
                                     poc[HD:HD + 1, :],
                                     start=True, stop=True)
                    rcb = sp.tile([64, QW], f32, tag="recb")
                    nc.vector.reciprocal_approx_fast(out=rcb[:], in_=dn[:])
                    nc.vector.tensor_tensor(
                        out=attnT[hp][qb][pb:pb + 64, :],
                        in0=poc[0:HD, :], in1=rcb[:], op=Mult)

                def emit_attention(qb, fillers, late_fillers=(),
                                   span_f=None):
                    """Waves over (hp, kt); scores(i+1)->exp(i)->filler->PV(i).

                    fillers: closures to splice in during early waves
                    late_fillers: spliced during the hp=1 half (block 3)
                    """
                    nsk = (qb + 1) * 4
                    # build the flat wave list
                    waves = [(hp, kt) for hp in range(2) for kt in range(nsk)]
                    nW = len(waves)
                    fillers = list(fillers)
                    late = list(late_fillers)
                    # early fillers must land in the first 3/4 of waves of
                    # each half so block-3 k/v projections finish before the
                    # diagonal waves need them
                    po_t = {}
                    sc_t = {}
                    prt_t = {}

                    def emit_scores(i):
                        hp, kt = waves[i]
                        if kt == 0:
                            for hh in range(2):
                                po_t[(hp, hh)] = psO.tile(
                                    [128, QW], f32, tag="pvacc",
                                    name=f"po_{hp}_{hh}_{qb}")
                        c0 = max(0, kt * 128 - qb * QW)
                        ps = psS.tile([128, 2 * QW], f32, tag="spair",
                                      name=f"sc_{hp}_{qb}_{kt}")
                        sc_t[i] = (ps, c0)
                        for hh in range(2):
                            pb = hh * 64
                            nc.tensor.matmul(
                                ps[:, hh * QW + c0:(hh + 1) * QW],
                                kTt[hp][kt // 4][pb:pb + 64,
                                                 (kt % 4) * 128:
                                                 (kt % 4) * 128 + 128],
                                qT[hp][qb][pb:pb + 64, c0:QW],
                                start=True, stop=True,
                                tile_position=(pb, 0))

                    def emit_exp(i):
                        hp, kt = waves[i]
                        ps, c0 = sc_t[i]
                        prt = wp.tile([128, 2 * QW], MMDT, tag="probs",
                                      name=f"pr_{hp}_{qb}_{kt}")
                        prt_t[i] = prt
                        nc.scalar.activation(prt[:], ps[:], Exp, scale=0.125)
                        if kt >= nsk - 4:
                            j = kt - (nsk - 4)
                            nc.gpsimd.affine_select(
                                out=prt[:].rearrange(
                                    "p (t f) -> p t f", f=QW),
                                in_=prt[:].rearrange(
                                    "p (t f) -> p t f", f=QW),
                                pattern=[[0, 2], [1, QW]], base=-128 * j,
                                channel_multiplier=-1,
                                compare_op=mybir.AluOpType.is_ge,
                                fill=0.0)

                    def emit_pv(i):
                        hp, kt = waves[i]
                        prt = prt_t.pop(i)
                        _, c0 = sc_t.pop(i)
                        for hh in range(2):
                            h = 2 * hp + hh
                            nc.tensor.matmul(
                                po_t[(hp, hh)][:, c0:QW],
                                v_sb[kt][:, h * (HD + 1):
                                         h * (HD + 1) + 128],
                                prt[:, hh * QW + c0:(hh + 1) * QW],
                                start=(kt == 0), stop=(kt == nsk - 1))
                        if kt == nsk - 1:
                            for hh in range(2):
                                poc = normalize_evac(
                                    hp, qb, hh, po_t.pop((hp, hh)))
                                deferred.setdefault(i + 2, []).append(
                                    lambda h1=hp, h2=hh, p=poc:
                                    normalize_tail(h1, qb, h2, p))

                    # software pipeline; filler granules are (cost_ns,
                    # closure), paced so cumulative filler PE-time tracks the
                    # wave index (fillers over the first span_f waves, late
                    # fillers over the first 3/4 of the hp=1 half). When the
                    # pools run dry mid-block, dummy warmers keep the PE duty
                    # cycle high enough that the HAM clock gate stays at 8/8.
                    if span_f is None:
                        span_f = max(1, (3 * nW) // 4)
                    span_l = max(1, (3 * nsk) // 4)
                    tot_f = sum(c for c, _ in fillers) or 1
                    tot_l = sum(c for c, _ in late) or 1
                    deferred = {}
                    emit_scores(0)
                    cf = cl = 0.0
                    fi = li = 0
                    for i in range(nW):
                        if i + 1 < nW:
                            emit_scores(i + 1)
                        emit_exp(i)
                        for g in deferred.pop(i, []):
                            g()
                        popped = 0
                        tgt = tot_f * min(i + 1, span_f) / span_f
                        while fi < len(fillers) and cf < tgt:
                            c, g = fillers[fi]
                            g()
                            cf += c
                            fi += 1
                            popped += 1
                        # late fillers start 2 waves into the hp=1 half so
                        # the hp=0 deferred normalize tails (which write
                        # attnT for this block) are emitted first
                        j = i - nsk - 2
                        if j >= 0 and late:
                            tgt = tot_l * min(j + 1, span_l) / span_l
                            while li < len(late) and cl < tgt:
                                c, g = late[li]
                                g()
                                cl += c
                                li += 1
                                popped += 1
                        if i > 0:
                            emit_pv(i - 1)
                    emit_pv(nW - 1)
                    for k in sorted(deferred):
                        for g in deferred.pop(k):
                            g()
                    for _, g in fillers[fi:] + late[li:]:
                        g()

                # ================= main schedule ==========================
                def p_granules(sb, which="qkv"):
                    # interleave chains: [c0g1 c0g2 c1g1 c1g2 c0g3 c2g1
                    # c2g2 c1g3 ...] so each rope matmul (g3) trails its
                    # gpsimd inputs by >=2 granules of other PE work and at
                    # most two psA "mix" slots are ever live
                    chains = []
                    if "q" in which:
                        for hp in range(2):
                            chains.append(qk_proj_granules(wq_s, qT, hp, sb))
                    if "k" in which:
                        for hp in range(2):
                            chains.append(qk_proj_granules(wk_s, kTt, hp, sb))
                    vg = []
                    if "v" in which:
                        vg = [(880, lambda s=st: emit_v(s))
                              for st in range(4 * sb, 4 * sb + 4)]
                    seq = []
                    prev3 = None
                    for ch in chains:
                        seq.append(ch[0])
                        if prev3 is not None:
                            seq.append(prev3)
                        prev3 = ch[1]
                    for i, g in enumerate(vg):
                        seq.append(g)
                        if i == 0 and prev3 is not None:
                            seq.append(prev3)
                            prev3 = None
                    if prev3 is not None:
                        seq.append(prev3)
                    return seq

                # P(0)
                for _, g in p_granules(0):
                    g()

                # A(0) with P(1) as filler
                emit_attention(0, p_granules(1))

                # A(1): P(2) as filler (out-projections deferred to later
                # blocks, which have no projection work left)
                emit_attention(1, p_granules(2))

                # A(2): P(3) q-projs + O(0) + O(1) as filler
                f = p_granules(3, "q")
                for st in range(0, 8):
                    f += out_granules(st)
                emit_attention(2, f)

                # A(3): P(3) k/v projs + O(2) during hp=0 (k/v done before
                # the diagonal waves need them); block-3 hp0-half outs
                # during the hp=1 waves
                f = p_granules(3, "kv")
                for st in range(8, 12):
                    f += out_granules(st)
                late = [(260, lambda s=st, d=db: emit_out_half0(s, d))
                        for st in range(12, 16) for db in range(2)]
                emit_attention(3, f, late, span_f=12)

                # tail: hp1 halves + add + store
                for st in range(12, 16):
                    emit_out_half1(st)

    nc.compile()
    return nc


def _shard_inputs(x, cos, sin, wq, wk, wv, wo):
    x = np.asarray(x, dtype=np.float32)
    cosT = np.asarray(cos, np.float32).reshape(S, HD).T
    sinT = np.asarray(sin, np.float32).reshape(S, HD).T
    cs2 = np.ascontiguousarray(
        np.concatenate([cosT, cosT], axis=0)).astype(np.float16)
    sn2 = np.ascontiguousarray(
        np.concatenate([sinT, sinT], axis=0)).astype(np.float16)

    def prep_w(w, rows):
        # [GC, D] row slice -> transpose -> [D, GC] -> per-128 k-chunks to
        # [128, KT*GC] so every on-device slice is a contiguous DMA
        wT = np.asarray(w, np.float32)[rows, :].T          # [D, GC]
        return np.ascontiguousarray(
            wT.reshape(KT, 128, GC).transpose(1, 0, 2).reshape(128, KT * GC)
        ).astype(np.float16)

    in_maps = []
    for c in range(NCORES):
        b, g = c // GROUPS, c % GROUPS
        rows = slice(g * GC, (g + 1) * GC)
        in_maps.append({
            "xT": np.ascontiguousarray(x[b].T).astype(np.float16),
            "wqT": prep_w(wq, rows),
            "wkT": prep_w(wk, rows),
            "wvT": prep_w(wv, rows),
            "woT": np.ascontiguousarray(
                np.asarray(wo, np.float32)[:, rows].T).astype(np.float16),
            "cs2": cs2,
            "sn2": sn2,
        })
    return in_maps


def _run(inputs, trace=False, trace_kwargs=None):
    if "nc" not in _cache:
        _cache["nc"] = _build()
    nc = _cache["nc"]
    in_maps = _shard_inputs(
        inputs["x"], inputs["cos"], inputs["sin"],
        inputs["wq"], inputs["wk"], inputs["wv"], inputs["wo"])
    res = run_bass_kernel_spmd(
        nc, in_maps, list(range(NCORES)), trace=trace,
        **(trace_kwargs or {}))
    full = np.zeros((B, S, D), dtype=np.float32)
    for c in range(NCORES):
        full[c // GROUPS] += res.results[c]["out"].astype(np.float32)
    return full, res


def kernel(**inputs):
    full, _ = _run(inputs, trace=False)
    return full


# revision 38
# speedup vs baseline: 1.1706x; 1.0158x over previous
"""Trainium2 Bass kernel for nn_Attention_84473416778449.

Reference computation (B=2, S=2048, D=1024, H=16, HD=64, fp32):
    q/k/v = x @ w{q,k,v}.T ; RoPE(q, k) ; causal softmax attention ; out @ wo.T

Sharding: 8 cores = (batch 2) x (head-group 4). Each core computes 4 heads of
one batch end-to-end and a partial output projection over its 256 channels;
the host sums the 4 partials per batch (in fp32; device emits fp16 partials).

v2 structure (vs the 222us baseline):
  * All inputs pre-cast to fp16 on the host; loads go out as fine-grained
    HWDGE DMAs (x + out stores on sync, weights/tables on scalar) so the
    first projection matmul issues ~2us in and converting-DMA slow paths
    are gone entirely.
  * Score matmuls for the two heads of a tile pair are packed into disjoint
    PE row-groups (tile_position (0,0)/(64,0)) and run concurrently.
  * Each attention wave = one key tile x head pair -> one [128,1024] PSUM
    group (2 banks); exp is a single merged ACTIVATE over both heads
    (halves ACT instruction-overhead); causal masking is one 2-D
    affine_select over the merged fp16 probs tile on gpsimd.
  * Waves are software-pipelined (scores(i+1) -> exp(i) -> filler -> PV(i))
    with the next block's projections and the previous block's output
    projections spliced in as PE filler, so the PE never idles long enough
    for the HAM clock gate to re-throttle.
  * RoPE: proj result is evacuated once to fp16 SBUF (DVE), the cos/sin
    multiplies run on gpsimd, the rotate-half runs on the PE (rt2 matmul),
    and only the final add stays on the DVE.
  * Softmax normalize uses reciprocal_approx_fast (1 DVE op, ~51 ULP).
  * The last block's output projections are split into head-pair halves so
    the hp0 half runs as filler inside the final attention block.
PSUM budget: scores 2x[128,1024] (4 banks) + PV accumulators 2x[128,512]
(2 banks) + proj/rope/out rotation 2x[128,512] (2 banks) = 8 banks.
"""
import sys

if "/opt/trn_rl_repo" not in sys.path:
    sys.path.insert(0, "/opt/trn_rl_repo")

import numpy as np

import concourse.bass as bass
import concourse.mybir as mybir
import concourse.tile as tile
from concourse import bacc
from concourse.bass_utils import run_bass_kernel_spmd

B, S, D, H, HD = 2, 2048, 1024, 16, 64
NCORES = 8
GROUPS = 4            # head groups
GH = H // GROUPS      # heads per group = 4
GC = GH * HD          # channels per group = 256
KT = D // 128         # 8 k-tiles over D
ST = S // 128         # 16 s-tiles
QB = 4                # sq blocks of 512
QW = S // QB          # 512
VW = GH * (HD + 1)    # 260: v tile payload columns

f32 = mybir.dt.float32
f16 = mybir.dt.float16
MMDT = mybir.dt.float16
Exp = mybir.ActivationFunctionType.Exp
Copy = mybir.ActivationFunctionType.Copy
Mult = mybir.AluOpType.mult
Add = mybir.AluOpType.add

_cache = {}


def _build():
    nc = bacc.Bacc("TRN2", num_devices=NCORES)

    xT = nc.dram_tensor("xT", [D, S], f16, kind="ExternalInput").ap()
    wqT = nc.dram_tensor("wqT", [128, KT * GC], f16, kind="ExternalInput").ap()
    wkT = nc.dram_tensor("wkT", [128, KT * GC], f16, kind="ExternalInput").ap()
    wvT = nc.dram_tensor("wvT", [128, KT * GC], f16, kind="ExternalInput").ap()
    woT = nc.dram_tensor("woT", [GC, D], f16, kind="ExternalInput").ap()
    cs2 = nc.dram_tensor("cs2", [128, S], f16, kind="ExternalInput").ap()
    sn2 = nc.dram_tensor("sn2", [128, S], f16, kind="ExternalInput").ap()
    out = nc.dram_tensor("out", [S, D], f16, kind="ExternalOutput").ap()

    with tile.TileContext(nc) as tc:
        with tc.tile_pool(name="persist", bufs=1) as pp, \
             tc.tile_pool(name="rope", bufs=3) as rp, \
             tc.tile_pool(name="probs", bufs=4) as wp, \
             tc.tile_pool(name="outsb", bufs=3) as op_, \
             tc.tile_pool(name="small", bufs=3) as sp:

            # ---- persistent SBUF tiles ------------------------------------
            # one [128, S] tile per k-tile: a single DMA loads all 4 blocks
            xTw = [pp.tile([128, S], MMDT, tag=f"xT{kt}", name=f"xTw{kt}")
                   for kt in range(KT)]
            xTr = [[xTw[kt][:, cb * QW:(cb + 1) * QW] for cb in range(QB)]
                   for kt in range(KT)]
            wq_s = pp.tile([128, KT * GC], MMDT, tag="wq", name="wq_s")
            wk_s = pp.tile([128, KT * GC], MMDT, tag="wk", name="wk_s")
            wv_s = pp.tile([128, KT * GC], MMDT, tag="wv", name="wv_s")
            wo_s = [pp.tile([128, D], MMDT, tag=f"wo{kt}", name=f"wo{kt}")
                    for kt in range(2)]
            cs_sb = pp.tile([128, S], f16, tag="cs")
            sn_sb = pp.tile([128, S], f16, tag="sn")

            # ---- loads: x k-tiles split across both HWDGE rings and
            # interleaved with the weight chunks in first-use order, so the
            # P(0) projection chains never starve on a single ring ----------
            def ld_x(eng, kt):
                eng.dma_start(xTw[kt][:], xT[kt * 128:(kt + 1) * 128, :])

            nc.scalar.dma_start(wq_s[:, 0:2 * GC], wqT[:, 0:2 * GC])
            for kt in (0, 1, 2, 3):
                ld_x(nc.sync, kt)
            ld_x(nc.scalar, 4)
            nc.scalar.dma_start(wq_s[:, 2 * GC:], wqT[:, 2 * GC:])
            ld_x(nc.scalar, 5)
            nc.sync.dma_start(cs_sb[:, 0:QW], cs2[:, 0:QW])
            nc.sync.dma_start(sn_sb[:, 0:QW], sn2[:, 0:QW])
            nc.scalar.dma_start(wk_s[:, 0:2 * GC], wkT[:, 0:2 * GC])
            ld_x(nc.sync, 6)
            nc.scalar.dma_start(wk_s[:, 2 * GC:], wkT[:, 2 * GC:])
            ld_x(nc.scalar, 7)
            nc.sync.dma_start(wv_s[:, 0:4 * GC], wvT[:, 0:4 * GC])
            nc.sync.dma_start(wv_s[:, 4 * GC:], wvT[:, 4 * GC:])
            nc.scalar.dma_start(cs_sb[:, QW:], cs2[:, QW:])
            nc.scalar.dma_start(sn_sb[:, QW:], sn2[:, QW:])
            for kt in range(2):
                nc.sync.dma_start(wo_s[kt][:], woT[kt * 128:(kt + 1) * 128, :])

            # ---- constants (rotate-half matrix, built fp32 -> cast) -------
            cscratch = pp.tile([128, 128], f32, tag="cscratch")
            nc.gpsimd.memset(cscratch[:], 0.0)
            for blk in range(2):
                sub = cscratch[blk * 64:(blk + 1) * 64,
                               blk * 64:(blk + 1) * 64]
                nc.gpsimd.affine_select(   # -1 where p - f == 32
                    out=sub, in_=sub, pattern=[[-1, 64]], base=-32,
                    channel_multiplier=1,
                    compare_op=mybir.AluOpType.not_equal, fill=-1.0)
                nc.gpsimd.affine_select(   # +1 where f - p == 32
                    out=sub, in_=sub, pattern=[[1, 64]], base=-32,
                    channel_multiplier=-1,
                    compare_op=mybir.AluOpType.not_equal, fill=1.0)
            rt2 = pp.tile([128, 128], MMDT, tag="rt2")
            nc.vector.tensor_copy(rt2[:], cscratch[:])

            ones65 = pp.tile([65, 64], MMDT, tag="ones65")
            nc.gpsimd.memset(ones65[:], 1.0)

            qT = [[pp.tile([128, QW], MMDT, tag=f"qT{i}_{b}",
                           name=f"qT{i}_{b}") for b in range(QB)]
                  for i in range(2)]
            kTt = [[pp.tile([128, QW], MMDT, tag=f"kT{i}_{b}",
                            name=f"kT{i}_{b}") for b in range(QB)]
                   for i in range(2)]
            attnT = [[pp.tile([128, QW], MMDT, tag=f"aT{i}_{b}",
                              name=f"aT{i}_{b}") for b in range(QB)]
                     for i in range(2)]
            v_sb = [pp.tile([128, VW + 64], MMDT, tag=f"v{i}",
                            name=f"v{i}") for i in range(ST)]
            # fp32 partials for the split last-block output projection
            ohalf = [pp.tile([128, QW], f32, tag=f"oh{i}", name=f"oh{i}")
                     for i in range(8)]

            with tc.tile_pool(name="psS", bufs=2, space="PSUM") as psS, \
                 tc.tile_pool(name="psO", bufs=2, space="PSUM") as psO, \
                 tc.tile_pool(name="psA", bufs=2, space="PSUM") as psA:

                def qk_proj_granules(w_src, dst, hp, sb):
                    # g12 is atomic: the full PSUM accumulation chain plus
                    # its evacuation stay inside one granule so no other
                    # psA "mix" allocation can rotate into the open slot.
                    # Only the rope matmul (g3) is split out, so it never
                    # head-of-line blocks the PE queue on the gpsimd
                    # multiplies it depends on.
                    cols = slice(sb * QW, (sb + 1) * QW)
                    st_ = {}

                    def g12():
                        pq = psA.tile([128, QW], f32, tag="mix",
                                      name=f"pq_{hp}_{sb}")
                        for kt in range(KT):
                            nc.tensor.matmul(
                                pq[:],
                                w_src[:, kt * GC + hp * 128:
                                      kt * GC + hp * 128 + 128],
                                xTr[kt][sb],
                                start=(kt == 0), stop=(kt == KT - 1))
                        qraw = rp.tile([128, QW], MMDT, tag="qraw")
                        nc.vector.tensor_copy(qraw[:], pq[:])
                        tsn = rp.tile([128, QW], MMDT, tag="tsn")
                        nc.gpsimd.tensor_tensor(
                            out=tsn[:], in0=qraw[:], in1=sn_sb[:, cols],
                            op=Mult)
                        tcs = rp.tile([128, QW], MMDT, tag="tcs")
                        nc.gpsimd.tensor_tensor(
                            out=tcs[:], in0=qraw[:], in1=cs_sb[:, cols],
                            op=Mult)
                        st_["rope"] = (tsn, tcs)

                    def g3():
                        tsn, tcs = st_.pop("rope")
                        pr = psA.tile([128, QW], f32, tag="mix",
                                      name=f"pr_{hp}_{sb}")
                        nc.tensor.matmul(pr[:], rt2[:], tsn[:],
                                         start=True, stop=True)
                        nc.vector.tensor_tensor(
                            out=dst[hp][sb][:], in0=pr[:], in1=tcs[:],
                            op=Add)

                    return [(1720, g12), (260, g3)]

                def emit_qk_proj(w_src, dst, hp, sb):
                    for _, g in qk_proj_granules(w_src, dst, hp, sb):
                        g()

                def emit_v(st):
                    pv = psA.tile([128, QW], f32, tag="mix",
                                  name=f"pv_{st}")
                    for kt in range(KT):
                        nc.tensor.matmul(
                            pv[:, 0:GC],
                            xTr[kt][st // 4][:, (st % 4) * 128:
                                             (st % 4) * 128 + 128],
                            wv_s[:, kt * GC:(kt + 1) * GC],
                            start=(kt == 0), stop=(kt == KT - 1))
                    vt = v_sb[st]
                    vhe = vt[:, 0:VW].rearrange("p (h e) -> p h e", e=HD + 1)
                    nc.gpsimd.memset(vt[:, VW:VW + 64], 0.0)
                    nc.gpsimd.memset(vhe[:, :, HD:HD + 1], 1.0)
                    nc.vector.tensor_copy(
                        vhe[:, :, 0:HD],
                        pv[:, 0:GC].rearrange("p (h d) -> p h d", d=HD))

                def out_granules(st):
                    # two granules (one per 512-col half); store rides on the
                    # second so the whole row goes out as one DMA
                    st_ = {}

                    def g(db):
                        if db == 0:
                            st_["ob"] = op_.tile([128, D], f16, tag="outsb",
                                                 name=f"ob_{st}")
                        ob = st_["ob"]
                        pc = psA.tile([128, QW], f32, tag="mix",
                                      name=f"pc_{st}_{db}")
                        for kt in range(2):
                            nc.tensor.matmul(
                                pc[:],
                                attnT[kt][st // 4][:, (st % 4) * 128:
                                                   (st % 4) * 128 + 128],
                                wo_s[kt][:, db * QW:(db + 1) * QW],
                                start=(kt == 0), stop=(kt == 1))
                        nc.vector.tensor_copy(
                            ob[:, db * QW:(db + 1) * QW], pc[:])
                        if db == 1:
                            st_.pop("ob")
                            nc.sync.dma_start(
                                out[st * 128:(st + 1) * 128, :], ob[:])

                    return [(480, lambda: g(0)), (520, lambda: g(1))]

                def emit_out(st):
                    for _, g in out_granules(st):
                        g()

                def warmer():
                    # dummy 512-col matmul: keeps the HAM clock gate at 8/8
                    # through ACT-paced stretches with no real filler left
                    pw = psA.tile([128, QW], f32, tag="mix", name="warm")
                    nc.tensor.matmul(pw[:], rt2[:], xTw[0][:, 0:QW],
                                     start=True, stop=True)

                def emit_out_half0(st, db):
                    # hp0 half-contraction of the block-3 output projection
                    pc = psA.tile([128, QW], f32, tag="mix",
                                  name=f"pc0_{st}_{db}")
                    nc.tensor.matmul(
                        pc[:],
                        attnT[0][st // 4][:, (st % 4) * 128:
                                          (st % 4) * 128 + 128],
                        wo_s[0][:, db * QW:(db + 1) * QW],
                        start=True, stop=True)
                    oh = ohalf[(st % 4) * 2 + db]
                    nc.vector.tensor_copy(oh[:], pc[:])

                def emit_out_half1(st):
                    ob = op_.tile([128, D], f16, tag="outsb")
                    for db in range(2):
                        pc = psA.tile([128, QW], f32, tag="mix",
                                      name=f"pc1_{st}_{db}")
                        nc.tensor.matmul(
                            pc[:],
                            attnT[1][st // 4][:, (st % 4) * 128:
                                              (st % 4) * 128 + 128],
                            wo_s[1][:, db * QW:(db + 1) * QW],
                            start=True, stop=True)
                        nc.vector.tensor_tensor(
                            out=ob[:, db * QW:(db + 1) * QW], in0=pc[:],
                            in1=ohalf[(st % 4) * 2 + db][:], op=Add)
                    nc.sync.dma_start(out[st * 128:(st + 1) * 128, :], ob[:])

                def normalize_evac(hp, qb, hh, po):
                    # one fp16 copy frees the psO slot immediately so the
                    # next half-block's PV chain is not blocked
                    poc = sp.tile([65, QW], MMDT, tag="poc",
                                  name=f"poc_{hp}_{qb}_{hh}")
                    nc.vector.tensor_copy(poc[:], po[0:HD + 1, :])
                    return poc

                def normalize_tail(hp, qb, hh, poc):
                    # denominator broadcast is a tiny K=1 PE matmul off
                    # poc's row 64; deferred ~2 waves so it never head-of-
                    # line blocks the PE queue on the DVE poc copy
                    pb = hh * 64
                    dn = psA.tile([64, QW], f32, tag="mix",
                                  name=f"dn_{hp}_{qb}_{hh}")
                    nc.tensor.matmul(dn[:], ones65[HD:HD + 1, :],
                                     poc[HD:HD + 1, :],
                                     start=True, stop=True)
                    rcb = sp.tile([64, QW], f32, tag="recb")
                    nc.vector.reciprocal_approx_fast(out=rcb[:], in_=dn[:])
                    nc.vector.tensor_tensor(
                        out=attnT[hp][qb][pb:pb + 64, :],
                        in0=poc[0:HD, :], in1=rcb[:], op=Mult)

                def emit_attention(qb, fillers, late_fillers=(),
                                   span_f=None):
                    """Waves over (hp, kt); scores(i+1)->exp(i)->filler->PV(i).

                    fillers: closures to splice in during early waves
                    late_fillers: spliced during the hp=1 half (block 3)
                    """
                    nsk = (qb + 1) * 4
                    # build the flat wave list
                    waves = [(hp, kt) for hp in range(2) for kt in range(nsk)]
                    nW = len(waves)
                    fillers = list(fillers)
                    late = list(late_fillers)
                    # early fillers must land in the first 3/4 of waves of
                    # each half so block-3 k/v projections finish before the
                    # diagonal waves need them
                    po_t = {}
                    sc_t = {}
                    prt_t = {}

                    def emit_scores(i):
                        hp, kt = waves[i]
                        if kt == 0:
                            for hh in range(2):
                                po_t[(hp, hh)] = psO.tile(
                                    [128, QW], f32, tag="pvacc",
                                    name=f"po_{hp}_{hh}_{qb}")
                        c0 = max(0, kt * 128 - qb * QW)
                        ps = psS.tile([128, 2 * QW], f32, tag="spair",
                                      name=f"sc_{hp}_{qb}_{kt}")
                        sc_t[i] = (ps, c0)
                        for hh in range(2):
                            pb = hh * 64
                            nc.tensor.matmul(
                                ps[:, hh * QW + c0:(hh + 1) * QW],
                                kTt[hp][kt // 4][pb:pb + 64,
                                                 (kt % 4) * 128:
                                                 (kt % 4) * 128 + 128],
                                qT[hp][qb][pb:pb + 64, c0:QW],
                                start=True, stop=True,
                                tile_position=(pb, 0))

                    def emit_exp(i):
                        hp, kt = waves[i]
                        ps, c0 = sc_t[i]
                        prt = wp.tile([128, 2 * QW], MMDT, tag="probs",
                                      name=f"pr_{hp}_{qb}_{kt}")
                        prt_t[i] = prt
                        nc.scalar.activation(prt[:], ps[:], Exp, scale=0.125)
                        if kt >= nsk - 4:
                            j = kt - (nsk - 4)
                            nc.gpsimd.affine_select(
                                out=prt[:].rearrange(
                                    "p (t f) -> p t f", f=QW),
                                in_=prt[:].rearrange(
                                    "p (t f) -> p t f", f=QW),
                                pattern=[[0, 2], [1, QW]], base=-128 * j,
                                channel_multiplier=-1,
                                compare_op=mybir.AluOpType.is_ge,
                                fill=0.0)

                    def emit_pv(i):
                        hp, kt = waves[i]
                        prt = prt_t.pop(i)
                        _, c0 = sc_t.pop(i)
                        for hh in range(2):
                            h = 2 * hp + hh
                            nc.tensor.matmul(
                                po_t[(hp, hh)][:, c0:QW],
                                v_sb[kt][:, h * (HD + 1):
                                         h * (HD + 1) + 128],
                                prt[:, hh * QW + c0:(hh + 1) * QW],
                                start=(kt == 0), stop=(kt == nsk - 1))
                        if kt == nsk - 1:
                            for hh in range(2):
                                poc = normalize_evac(
                                    hp, qb, hh, po_t.pop((hp, hh)))
                                deferred.setdefault(i + 2, []).append(
                                    lambda h1=hp, h2=hh, p=poc:
                                    normalize_tail(h1, qb, h2, p))

                    # software pipeline; filler granules are (cost_ns,
                    # closure), paced so cumulative filler PE-time tracks the
                    # wave index (fillers over the first span_f waves, late
                    # fillers over the first 3/4 of the hp=1 half). When the
                    # pools run dry mid-block, dummy warmers keep the PE duty
                    # cycle high enough that the HAM clock gate stays at 8/8.
                    if span_f is None:
                        span_f = max(1, (3 * nW) // 4)
                    span_l = max(1, (3 * nsk) // 4)
                    tot_f = sum(c for c, _ in fillers) or 1
                    tot_l = sum(c for c, _ in late) or 1
                    deferred = {}
                    emit_scores(0)
                    cf = cl = 0.0
                    fi = li = 0
                    for i in range(nW):
                        if i + 1 < nW:
                            emit_scores(i + 1)
                        emit_exp(i)
                        for g in deferred.pop(i, []):
                            g()
                        popped = 0
                        tgt = tot_f * min(i + 1, span_f) / span_f
                        while fi < len(fillers) and cf < tgt:
                            c, g = fillers[fi]
                            g()
                            cf += c
                            fi += 1
                            popped += 1
                        # late fillers start 2 waves into the hp=1 half so
                        # the hp=0 deferred normalize tails (which write
                        # attnT for this block) are emitted first
                        j = i - nsk - 2
                        if j >= 0 and late:
                            tgt = tot_l * min(j + 1, span_l) / span_l
                            while li < len(late) and cl < tgt:
                                c, g = late[li]
                                g()
                                cl += c
                                li += 1
                                popped += 1
                        if qb == 3 and i >= nsk and popped == 0:
                            # the final half has no real filler left: dummy
                            # warmers keep the HAM clock at 8/8 so the tail
                            # does not run at half rate
                            warmer()
                            warmer()
                        if i > 0:
                            emit_pv(i - 1)
                    emit_pv(nW - 1)
                    for k in sorted(deferred):
                        for g in deferred.pop(k):
                            g()
                    for _, g in fillers[fi:] + late[li:]:
                        g()

                # ================= main schedule ==========================
                def p_granules(sb, which="qkv"):
                    # interleave chains: [c0g1 c0g2 c1g1 c1g2 c0g3 c2g1
                    # c2g2 c1g3 ...] so each rope matmul (g3) trails its
                    # gpsimd inputs by >=2 granules of other PE work and at
                    # most two psA "mix" slots are ever live
                    chains = []
                    if "q" in which:
                        for hp in range(2):
                            chains.append(qk_proj_granules(wq_s, qT, hp, sb))
                    if "k" in which:
                        for hp in range(2):
                            chains.append(qk_proj_granules(wk_s, kTt, hp, sb))
                    vg = []
                    if "v" in which:
                        vg = [(880, lambda s=st: emit_v(s))
                              for st in range(4 * sb, 4 * sb + 4)]
                    seq = []
                    prev3 = None
                    for ch in chains:
                        seq.append(ch[0])
                        if prev3 is not None:
                            seq.append(prev3)
                        prev3 = ch[1]
                    for i, g in enumerate(vg):
                        seq.append(g)
                        if i == 0 and prev3 is not None:
                            seq.append(prev3)
                            prev3 = None
                    if prev3 is not None:
                        seq.append(prev3)
                    return seq

                # P(0)
                for _, g in p_granules(0):
                    g()

                # A(0) with P(1) as filler
                emit_attention(0, p_granules(1))

                # A(1): P(2) as filler (out-projections deferred to later
                # blocks, which have no projection work left)
                emit_attention(1, p_granules(2))

                # A(2): P(3) q-projs + O(0) + O(1) as filler
                f = p_granules(3, "q")
                for st in range(0, 8):
                    f += out_granules(st)
                emit_attention(2, f)

                # A(3): P(3) k/v projs + O(2) during hp=0 (k/v done before
                # the diagonal waves need them); block-3 hp0-half outs
                # during the hp=1 waves
                f = p_granules(3, "kv")
                for st in range(8, 12):
                    f += out_granules(st)
                late = [(260, lambda s=st, d=db: emit_out_half0(s, d))
                        for st in range(12, 16) for db in range(2)]
                emit_attention(3, f, late, span_f=12)

                # tail: hp1 halves + add + store
                for st in range(12, 16):
                    emit_out_half1(st)

    nc.compile()
    return nc


def _shard_inputs(x, cos, sin, wq, wk, wv, wo):
    x = np.asarray(x, dtype=np.float32)
    cosT = np.asarray(cos, np.float32).reshape(S, HD).T
    sinT = np.asarray(sin, np.float32).reshape(S, HD).T
    cs2 = np.ascontiguousarray(
        np.concatenate([cosT, cosT], axis=0)).astype(np.float16)
    sn2 = np.ascontiguousarray(
        np.concatenate([sinT, sinT], axis=0)).astype(np.float16)

    def prep_w(w, rows):
        # [GC, D] row slice -> transpose -> [D, GC] -> per-128 k-chunks to
        # [128, KT*GC] so every on-device slice is a contiguous DMA
        wT = np.asarray(w, np.float32)[rows, :].T          # [D, GC]
        return np.ascontiguousarray(
            wT.reshape(KT, 128, GC).transpose(1, 0, 2).reshape(128, KT * GC)
        ).astype(np.float16)

    in_maps = []
    for c in range(NCORES):
        b, g = c // GROUPS, c % GROUPS
        rows = slice(g * GC, (g + 1) * GC)
        in_maps.append({
            "xT": np.ascontiguousarray(x[b].T).astype(np.float16),
            "wqT": prep_w(wq, rows),
            "wkT": prep_w(wk, rows),
            "wvT": prep_w(wv, rows),
            "woT": np.ascontiguousarray(
                np.asarray(wo, np.float32)[:, rows].T).astype(np.float16),
            "cs2": cs2,
            "sn2": sn2,
        })
    return in_maps


def _run(inputs, trace=False, trace_kwargs=None):
    if "nc" not in _cache:
        _cache["nc"] = _build()
    nc = _cache["nc"]
    in_maps = _shard_inputs(
        inputs["x"], inputs["cos"], inputs["sin"],
        inputs["wq"], inputs["wk"], inputs["wv"], inputs["wo"])
    res = run_bass_kernel_spmd(
        nc, in_maps, list(range(NCORES)), trace=trace,
        **(trace_kwargs or {}))
    full = np.zeros((B, S, D), dtype=np.float32)
    for c in range(NCORES):
        full[c // GROUPS] += res.results[c]["out"].astype(np.float32)
    return full, res


def kernel(**inputs):
    full, _ = _run(inputs, trace=False)
    return full
